# revision 5
# baseline (speedup 1.0000x reference)
"""MoE (top-2 of 8 experts, GELU MLP) on 8 Trainium2 NeuronCores.

Sharding: expert-parallel, one expert per core (hint: "shard W1/W2 along the
expert axis across M devices with all-to-all token dispatch/combine").
The host plays the role of the all-to-all fabric: it routes tokens (softmax
top-2 over the gate logits, computed in float64 -- the reference's selection
margins are >>fp32 noise so selection is exact), gathers each expert's tokens
to a padded capacity, and after the device pass combines the two expert
outputs per token with the routing weights. All heavy compute (>99.9% of
FLOPs: both 4096-wide GEMMs + exact erf-GELU) runs on the NeuronCores in
fp32r (full PE rate, ~1e-4 rms error vs fp32).

Device kernel per core e (SPMD, same program, different data):
    y = gelu(x_e @ W1[e]) @ W2[e]
with x_e fed transposed ([C, cap]) so GEMM1 produces h^T directly
(stationary = W1 tiles) and GEMM2 (stationary = h^T tiles, moving = W2
panels) produces token-major y without any on-device transposes.
"""

import sys

if "/opt/trn_rl_repo" not in sys.path:
    sys.path.insert(0, "/opt/trn_rl_repo")

import numpy as np

import concourse.bass as bass  # noqa: F401  (registers engine types)
import concourse.mybir as mybir
import concourse.tile as tile
from concourse import bacc
from concourse.bass_utils import run_bass_kernel_spmd

N_CORES = 8
C = 1024          # n_embd
E = 8             # n_experts
F = 4096          # d_ff
TOP_K = 2
KC = C // 128     # 8 k-tiles for GEMM1
KF = F // 128     # 32 k-tiles for GEMM2
F32 = mybir.dt.float32
F32R = mybir.dt.float32r

LAST_EXEC_TIME_NS = None      # set when tracing is enabled (see test harness)
LAST_RESULTS = None


def _install_axon_ntff_shim():
    """This image's `antenv` lacks `axon_hooks`; polyfill it so
    run_bass_kernel_spmd(trace=True) (or env BASS_TRACE=1) works instead of
    crashing on import. Registers the real ctypes NTFF hook when available."""
    import types

    try:
        import antenv
    except ImportError:
        return
    if hasattr(antenv, "axon_hooks"):
        return
    mod = types.ModuleType("antenv.axon_hooks")
    mod._hook = None

    def set_axon_ntff_profile_hook(h):
        mod._hook = h

    def get_axon_ntff_profile_hook():
        return mod._hook

    mod.set_axon_ntff_profile_hook = set_axon_ntff_profile_hook
    mod.get_axon_ntff_profile_hook = get_axon_ntff_profile_hook
    sys.modules["antenv.axon_hooks"] = mod
    antenv.axon_hooks = mod
    try:
        from trn_agent_boot.trn_boot import _ntff_profile_via_ctypes

        mod.set_axon_ntff_profile_hook(
            _ntff_profile_via_ctypes("/opt/axon/libaxon_pjrt.so")
        )
    except Exception:
        pass


_install_axon_ntff_shim()


def _chunks_for(maxcnt: int) -> list[int]:
    """Token chunks: each in {256, 384, 512} (PSUM bank = 512 fp32; fp32r
    needs moving dim >= 256 for full PE rate), multiples of 128, summing to
    >= maxcnt with minimal padding."""
    cap = max(256, -(-maxcnt // 128) * 128)
    chunks = []
    rem = cap
    while rem > 512:
        if rem == 640:               # avoid a trailing chunk < 256
            chunks.extend([384, 256])
            return chunks
        if rem == 768:
            chunks.extend([384, 384])
            return chunks
        chunks.append(512)
        rem -= 512
    chunks.append(max(rem, 256))
    assert all(c % 128 == 0 and 256 <= c <= 512 for c in chunks)
    return chunks


def _build(chunks: list[int]):
    """Build the per-core Bass program: y[cap, C] = gelu(xT.T @ W1) @ W2."""
    cap = sum(chunks)
    nc = bacc.Bacc("TRN2", target_bir_lowering=False, debug=False)
    xT = nc.dram_tensor("xT", [C, cap], F32R, kind="ExternalInput")
    w1 = nc.dram_tensor("w1", [C, F], F32R, kind="ExternalInput")
    w2 = nc.dram_tensor("w2", [F, C], F32R, kind="ExternalInput")
    y = nc.dram_tensor("y", [cap, C], F32, kind="ExternalOutput")
    gelu = mybir.ActivationFunctionType.Gelu

    with tile.TileContext(nc) as tc:
        with (
            tc.tile_pool(name="xp", bufs=1) as xp,
            tc.tile_pool(name="w1p", bufs=2) as w1p,
            tc.tile_pool(name="w2rp", bufs=1) as w2rp,
            tc.tile_pool(name="w2sp", bufs=4) as w2sp,
            tc.tile_pool(name="hp", bufs=1) as hp,
            tc.tile_pool(name="yp", bufs=4) as yp,
            tc.tile_pool(name="ps1", bufs=2, space="PSUM") as ps1,
            tc.tile_pool(name="ps2", bufs=2, space="PSUM") as ps2,
            tc.tile_pool(name="ps3", bufs=1, space="PSUM") as ps3,
        ):
            # W2 first half [F, 0:512] stays resident across all chunks
            # (loaded once, 8 MB); the second half streams per chunk.
            w2r = w2rp.tile([128, KF * 512], F32R, tag="w2r")
            for q in range(4):
                nc.sync.dma_start(
                    out=w2r[:, q * 8 * 512:(q + 1) * 8 * 512].rearrange(
                        "p (k c) -> p k c", k=8
                    ),
                    in_=w2.ap()[q * 1024:(q + 1) * 1024, 0:512].rearrange(
                        "(k p) c -> p k c", p=128
                    ),
                )

            t0 = 0
            for tn in chunks:
                nm = tn // 128
                # ---- load this chunk's tokens: [C, tn] -> [128, KC, tn]
                x_sb = xp.tile([128, KC * tn], F32R, tag="x")
                nc.sync.dma_start(
                    out=x_sb[:].rearrange("p (k n) -> p k n", k=KC),
                    in_=xT.ap()[:, t0:t0 + tn].rearrange("(k p) n -> p k n", p=128),
                )

                # ---- GEMM1 + GELU: hT[f, tokens] = gelu(W1.T @ x)
                h_tiles = []
                for fo in range(F // 256):
                    w1_sb = w1p.tile([128, KC * 256], F32R, tag="w1")
                    nc.sync.dma_start(
                        out=w1_sb[:].rearrange("p (k f) -> p k f", k=KC),
                        in_=w1.ap()[:, fo * 256:(fo + 1) * 256].rearrange(
                            "(k p) f -> p k f", p=128
                        ),
                    )
                    for fi in range(2):
                        ph = ps1.tile([128, tn], F32, tag="ph")
                        for k in range(KC):
                            nc.tensor.matmul(
                                ph[:],
                                lhsT=w1_sb[:, k * 256 + fi * 128:k * 256 + (fi + 1) * 128],
                                rhs=x_sb[:, k * tn:(k + 1) * tn],
                                start=(k == 0),
                                stop=(k == KC - 1),
                            )
                        hT = hp.tile([128, tn], F32R, tag=f"h{fo * 2 + fi}")
                        nc.scalar.activation(hT[:], ph[:], gelu)
                        h_tiles.append(hT)

                # ---- GEMM2 half 0 (resident W2): y[tok, 0:512]
                for m in range(nm):
                    py = ps2.tile([128, 512], F32, tag="py")
                    for k in range(KF):
                        nc.tensor.matmul(
                            py[:],
                            lhsT=h_tiles[k][:, m * 128:(m + 1) * 128],
                            rhs=w2r[:, k * 512:(k + 1) * 512],
                            start=(k == 0),
                            stop=(k == KF - 1),
                        )
                    y_sb = yp.tile([128, 512], F32, tag="y")
                    nc.vector.tensor_copy(y_sb[:], py[:])
                    nc.sync.dma_start(
                        out=y.ap()[t0 + m * 128:t0 + (m + 1) * 128, 0:512],
                        in_=y_sb[:],
                    )

                # ---- GEMM2 half 1 (streamed W2 k-panels): y[tok, 512:1024]
                pys = [
                    ps3.tile([128, 512], F32, tag=f"py1m{m}", name=f"py1m{m}")
                    for m in range(nm)
                ]
                for k in range(KF):
                    w2t = w2sp.tile([128, 512], F32R, tag="w2s")
                    nc.sync.dma_start(
                        out=w2t[:],
                        in_=w2.ap()[k * 128:(k + 1) * 128, 512:1024],
                    )
                    for m in range(nm):
                        nc.tensor.matmul(
                            pys[m][:],
                            lhsT=h_tiles[k][:, m * 128:(m + 1) * 128],
                            rhs=w2t[:],
                            start=(k == 0),
                            stop=(k == KF - 1),
                        )
                for m in range(nm):
                    y_sb = yp.tile([128, 512], F32, tag="y")
                    nc.vector.tensor_copy(y_sb[:], pys[m][:])
                    nc.sync.dma_start(
                        out=y.ap()[t0 + m * 128:t0 + (m + 1) * 128, 512:1024],
                        in_=y_sb[:],
                    )
                t0 += tn
    nc.compile()
    return nc


def kernel(x, Wg, W1, W2):
    global LAST_EXEC_TIME_NS, LAST_RESULTS
    x = np.asarray(x, dtype=np.float32)
    Wg = np.asarray(Wg, dtype=np.float32)
    W1 = np.asarray(W1, dtype=np.float32)
    W2 = np.asarray(W2, dtype=np.float32)
    B, T, _ = x.shape
    ntok = B * T
    xf = x.reshape(ntok, C)

    # ---- router (replicated gate, fp64 for stable selection)
    logits = xf.astype(np.float64) @ Wg.astype(np.float64)
    logits -= logits.max(-1, keepdims=True)
    probs = np.exp(logits)
    probs /= probs.sum(-1, keepdims=True)
    top2 = np.argsort(-probs, axis=-1, kind="stable")[:, :TOP_K]       # [ntok, 2]
    w12 = np.take_along_axis(probs, top2, axis=-1)
    w12 = w12 / w12.sum(-1, keepdims=True)                             # [ntok, 2]

    # aux load-balancing loss
    f_frac = np.bincount(top2.ravel(), minlength=E) / (ntok * TOP_K)
    P_mean = probs.mean(axis=0)
    aux_loss = np.float32(E * (f_frac * P_mean).sum())

    # ---- dispatch: gather each expert's tokens, pad to shared capacity
    token_lists = [np.nonzero((top2 == e).any(-1))[0] for e in range(E)]
    maxcnt = max(len(t) for t in token_lists)
    chunks = _chunks_for(maxcnt)
    cap = sum(chunks)

    in_maps = []
    for e in range(E):
        tl = token_lists[e]
        xe = np.zeros((C, cap), np.float32)
        xe[:, :len(tl)] = xf[tl].T
        in_maps.append({
            "xT": xe,
            "w1": np.ascontiguousarray(W1[e]),
            "w2": np.ascontiguousarray(W2[e]),
        })

    nc = _build(chunks)
    res = run_bass_kernel_spmd(nc, in_maps, list(range(N_CORES)))
    LAST_EXEC_TIME_NS = res.exec_time_ns
    LAST_RESULTS = res

    # ---- combine: out[t] = sum_k w12[t,k] * y_{expert k}[t]
    out = np.zeros((ntok, C), np.float64)
    for e in range(E):
        tl = token_lists[e]
        ye = res.results[e]["y"][:len(tl)].astype(np.float64)
        we = np.where(top2[tl, 0] == e, w12[tl, 0], w12[tl, 1])[:, None]
        out[tl] += we * ye
    return out.reshape(B, T, C).astype(np.float32), aux_loss


# revision 9
# speedup vs baseline: 1.1556x; 1.1556x over previous
"""MoE (top-2 of 8 experts, GELU MLP) on 8 Trainium2 NeuronCores.

Sharding: expert-parallel, one expert per core (hint: "shard W1/W2 along the
expert axis across M devices with all-to-all token dispatch/combine").
The host plays the role of the all-to-all fabric: it routes tokens (softmax
top-2 over the gate logits, computed in float64 -- the reference's selection
margins are >>fp32 noise so selection is exact), gathers each expert's tokens
to a padded capacity, and after the device pass combines the two expert
outputs per token with the routing weights. All heavy compute (>99.9% of
FLOPs: both 4096-wide GEMMs + exact erf-GELU) runs on the NeuronCores in
fp32r (full PE rate, ~1e-4 rms error vs fp32).

Device kernel per core e (SPMD, same program, different data):
    y = gelu(x_e @ W1[e]) @ W2[e]
with x_e fed transposed ([C, cap]) so GEMM1 produces h^T directly
(stationary = W1 tiles) and GEMM2 (stationary = h^T tiles, moving = W2
panels) produces token-major y without any on-device transposes.
"""

import sys

if "/opt/trn_rl_repo" not in sys.path:
    sys.path.insert(0, "/opt/trn_rl_repo")

import numpy as np

import concourse.bass as bass  # noqa: F401  (registers engine types)
import concourse.mybir as mybir
import concourse.tile as tile
from concourse import bacc
from concourse.bass_utils import run_bass_kernel_spmd

N_CORES = 8
C = 1024          # n_embd
E = 8             # n_experts
F = 4096          # d_ff
TOP_K = 2
KC = C // 128     # 8 k-tiles for GEMM1
KF = F // 128     # 32 k-tiles for GEMM2
F32 = mybir.dt.float32
F32R = mybir.dt.float32r

LAST_EXEC_TIME_NS = None      # set when tracing is enabled (see test harness)
LAST_RESULTS = None


def _install_axon_ntff_shim():
    """This image's `antenv` lacks `axon_hooks`; polyfill it so
    run_bass_kernel_spmd(trace=True) (or env BASS_TRACE=1) works instead of
    crashing on import. Registers the real ctypes NTFF hook when available."""
    import types

    try:
        import antenv
    except ImportError:
        return
    if hasattr(antenv, "axon_hooks"):
        return
    mod = types.ModuleType("antenv.axon_hooks")
    mod._hook = None

    def set_axon_ntff_profile_hook(h):
        mod._hook = h

    def get_axon_ntff_profile_hook():
        return mod._hook

    mod.set_axon_ntff_profile_hook = set_axon_ntff_profile_hook
    mod.get_axon_ntff_profile_hook = get_axon_ntff_profile_hook
    sys.modules["antenv.axon_hooks"] = mod
    antenv.axon_hooks = mod
    try:
        from trn_agent_boot.trn_boot import _ntff_profile_via_ctypes

        mod.set_axon_ntff_profile_hook(
            _ntff_profile_via_ctypes("/opt/axon/libaxon_pjrt.so")
        )
    except Exception:
        pass


_install_axon_ntff_shim()


def _chunks_for(maxcnt: int) -> list[int]:
    """Token chunks: each in {256, 384, 512} (PSUM bank = 512 fp32; fp32r
    needs moving dim >= 256 for full PE rate), multiples of 128, summing to
    >= maxcnt with minimal padding."""
    cap = max(256, -(-maxcnt // 128) * 128)
    q = cap // 128                   # 128-token tiles
    n = -(-q // 3)                   # chunks of <= 3 tiles (384: SBUF budget)
    chunks = [(q // n + (1 if i < q % n else 0)) * 128 for i in range(n)]
    assert sum(chunks) == cap
    assert all(c % 128 == 0 and 256 <= c <= 384 for c in chunks), chunks
    return chunks


def _build(chunks: list[int]):
    """Build the per-core Bass program: y[cap, C] = gelu(xT.T @ W1) @ W2."""
    cap = sum(chunks)
    nc = bacc.Bacc("TRN2", target_bir_lowering=False, debug=False)
    xT = nc.dram_tensor("xT", [C, cap], F32R, kind="ExternalInput")
    w1 = nc.dram_tensor("w1", [C, F], F32R, kind="ExternalInput")
    w2 = nc.dram_tensor("w2", [F, C], F32R, kind="ExternalInput")
    y = nc.dram_tensor("y", [cap, C], F32, kind="ExternalOutput")
    gelu = mybir.ActivationFunctionType.Gelu

    with tile.TileContext(nc) as tc:
        with (
            tc.tile_pool(name="xp", bufs=2) as xp,
            tc.tile_pool(name="w1p", bufs=4) as w1p,
            tc.tile_pool(name="w2rp", bufs=1) as w2rp,
            tc.tile_pool(name="w2sp", bufs=8) as w2sp,
            tc.tile_pool(name="hp", bufs=1) as hp,
            tc.tile_pool(name="yp", bufs=4) as yp,
            tc.tile_pool(name="ps1", bufs=3, space="PSUM") as ps1,
            tc.tile_pool(name="ps2", bufs=2, space="PSUM") as ps2,
            tc.tile_pool(name="ps3", bufs=1, space="PSUM") as ps3,
        ):
            # W2 first half [F, 0:512] stays resident across all chunks
            # (loaded once, 8 MB); the second half streams per chunk. The
            # load is emitted after chunk 0's GEMM1 so it doesn't starve
            # the pipeline-filling x/W1 DMAs at kernel start.
            w2r = w2rp.tile([128, KF * 512], F32R, tag="w2r")

            def load_w2r():
                for q in range(8):
                    nc.sync.dma_start(
                        out=w2r[:, q * 4 * 512:(q + 1) * 4 * 512].rearrange(
                            "p (k c) -> p k c", k=4
                        ),
                        in_=w2.ap()[q * 512:(q + 1) * 512, 0:512].rearrange(
                            "(k p) c -> p k c", p=128
                        ),
                    )

            t0 = 0
            for ci, tn in enumerate(chunks):
                nm = tn // 128
                # ---- load this chunk's tokens: [C, tn] -> [128, KC, tn]
                x_sb = xp.tile([128, KC * tn], F32R, tag="x")
                nc.sync.dma_start(
                    out=x_sb[:].rearrange("p (k n) -> p k n", k=KC),
                    in_=xT.ap()[:, t0:t0 + tn].rearrange("(k p) n -> p k n", p=128),
                )

                # ---- GEMM1 + GELU: hT[f, tokens] = gelu(W1.T @ x)
                h_tiles = []
                for fo in range(F // 256):
                    w1_sb = w1p.tile([128, KC * 256], F32R, tag="w1")
                    nc.sync.dma_start(
                        out=w1_sb[:].rearrange("p (k f) -> p k f", k=KC),
                        in_=w1.ap()[:, fo * 256:(fo + 1) * 256].rearrange(
                            "(k p) f -> p k f", p=128
                        ),
                    )
                    for fi in range(2):
                        ph = ps1.tile([128, tn], F32, tag="ph")
                        for k in range(KC):
                            nc.tensor.matmul(
                                ph[:],
                                lhsT=w1_sb[:, k * 256 + fi * 128:k * 256 + (fi + 1) * 128],
                                rhs=x_sb[:, k * tn:(k + 1) * tn],
                                start=(k == 0),
                                stop=(k == KC - 1),
                            )
                        hT = hp.tile([128, tn], F32R, tag=f"h{fo * 2 + fi}")
                        nc.scalar.activation(hT[:], ph[:], gelu)
                        h_tiles.append(hT)

                if ci == 0:
                    load_w2r()

                # ---- GEMM2 half 0 (resident W2): y[tok, 0:512]
                for m in range(nm):
                    py = ps2.tile([128, 512], F32, tag="py")
                    for k in range(KF):
                        nc.tensor.matmul(
                            py[:],
                            lhsT=h_tiles[k][:, m * 128:(m + 1) * 128],
                            rhs=w2r[:, k * 512:(k + 1) * 512],
                            start=(k == 0),
                            stop=(k == KF - 1),
                        )
                    y_sb = yp.tile([128, 512], F32, tag="y")
                    nc.vector.tensor_copy(y_sb[:], py[:])
                    nc.sync.dma_start(
                        out=y.ap()[t0 + m * 128:t0 + (m + 1) * 128, 0:512],
                        in_=y_sb[:],
                    )

                # ---- GEMM2 half 1 (streamed W2 k-panels): y[tok, 512:1024]
                pys = [
                    ps3.tile([128, 512], F32, tag=f"py1m{m}", name=f"py1m{m}")
                    for m in range(nm)
                ]
                for k in range(KF):
                    w2t = w2sp.tile([128, 512], F32R, tag="w2s")
                    nc.sync.dma_start(
                        out=w2t[:],
                        in_=w2.ap()[k * 128:(k + 1) * 128, 512:1024],
                    )
                    for m in range(nm):
                        nc.tensor.matmul(
                            pys[m][:],
                            lhsT=h_tiles[k][:, m * 128:(m + 1) * 128],
                            rhs=w2t[:],
                            start=(k == 0),
                            stop=(k == KF - 1),
                        )
                for m in range(nm):
                    y_sb = yp.tile([128, 512], F32, tag="y")
                    nc.vector.tensor_copy(y_sb[:], pys[m][:])
                    nc.sync.dma_start(
                        out=y.ap()[t0 + m * 128:t0 + (m + 1) * 128, 512:1024],
                        in_=y_sb[:],
                    )
                t0 += tn
    nc.compile()
    return nc


def kernel(x, Wg, W1, W2):
    global LAST_EXEC_TIME_NS, LAST_RESULTS
    x = np.asarray(x, dtype=np.float32)
    Wg = np.asarray(Wg, dtype=np.float32)
    W1 = np.asarray(W1, dtype=np.float32)
    W2 = np.asarray(W2, dtype=np.float32)
    B, T, _ = x.shape
    ntok = B * T
    xf = x.reshape(ntok, C)

    # ---- router (replicated gate, fp64 for stable selection)
    logits = xf.astype(np.float64) @ Wg.astype(np.float64)
    logits -= logits.max(-1, keepdims=True)
    probs = np.exp(logits)
    probs /= probs.sum(-1, keepdims=True)
    top2 = np.argsort(-probs, axis=-1, kind="stable")[:, :TOP_K]       # [ntok, 2]
    w12 = np.take_along_axis(probs, top2, axis=-1)
    w12 = w12 / w12.sum(-1, keepdims=True)                             # [ntok, 2]

    # aux load-balancing loss
    f_frac = np.bincount(top2.ravel(), minlength=E) / (ntok * TOP_K)
    P_mean = probs.mean(axis=0)
    aux_loss = np.float32(E * (f_frac * P_mean).sum())

    # ---- dispatch: gather each expert's tokens, pad to shared capacity
    token_lists = [np.nonzero((top2 == e).any(-1))[0] for e in range(E)]
    maxcnt = max(len(t) for t in token_lists)
    chunks = _chunks_for(maxcnt)
    cap = sum(chunks)

    in_maps = []
    for e in range(E):
        tl = token_lists[e]
        xe = np.zeros((C, cap), np.float32)
        xe[:, :len(tl)] = xf[tl].T
        in_maps.append({
            "xT": xe,
            "w1": np.ascontiguousarray(W1[e]),
            "w2": np.ascontiguousarray(W2[e]),
        })

    nc = _build(chunks)
    res = run_bass_kernel_spmd(nc, in_maps, list(range(N_CORES)))
    LAST_EXEC_TIME_NS = res.exec_time_ns
    LAST_RESULTS = res

    # ---- combine: out[t] = sum_k w12[t,k] * y_{expert k}[t]
    out = np.zeros((ntok, C), np.float64)
    for e in range(E):
        tl = token_lists[e]
        ye = res.results[e]["y"][:len(tl)].astype(np.float64)
        we = np.where(top2[tl, 0] == e, w12[tl, 0], w12[tl, 1])[:, None]
        out[tl] += we * ye
    return out.reshape(B, T, C).astype(np.float32), aux_loss
